# revision 33
# baseline (speedup 1.0000x reference)
"""DistMult scoring kernel for Trainium2 (8 NeuronCores, data-parallel).

score[b] = sum_d src[b,d] * rel[d] * dst[b,d],  rel = M[rel_idx]

Sharding: batch dim split evenly across 8 cores; rel row replicated.
Per-core dataflow ("colmajor" layout: partition p owns the contiguous row
span [p*C, (p+1)*C), C = rows//128, so every DMA descriptor run is
g*512B contiguous per partition and the output is ONE contiguous DMA):
  - src and dst stream in as [128, g=8, 128] tiles via SWDGE (gpsimd)
    DMAs that CAST fp32->bf16 in flight: HBM reads are the mandatory
    64MB fp32, SBUF writes halve, and every DVE op downstream runs in
    2x packed mode.
  - VectorE pass 1 (in place): s = s * rel_bcast (bf16, 2x_1p)
  - VectorE pass 2 (in place): s = s * dst      (bf16, 2x_1p)
  - VectorE tensor_reduce per group -> bf16 scores (2x; an fp32 scores
    output drops the whole reduce to 1x and costs +30us end to end)
  - one DVE copy upcasts scores bf16->f32 (~300ns), then a single
    contiguous output DMA on the scalar HWDGE ring.
DVE totals ~1.8us per 8-block group vs the ~2.95us DMA pace, so the
kernel runs at the DMA rate: measured ~181us/core vs the 179us HBM
roofline (64MB in @ ~358GB/s/core), 7.6% faster than the best fp32
pipeline (195.7us). bf16 rounding costs 6.3e-3 max-rel-err vs the 2e-2
gate (products round 3x at 2^-9 each, the sum accumulates in fp32
internally and rounds once more at the bf16 scores write).
"""

import numpy as np

N_CORES = 8
B = 500000
D = 128
P = 128
ROWS = B // N_CORES  # 62500 rows per core
GROUP = 8  # 128-row blocks per DMA group (~0.5MB per tensor per group)

# Kernel-structure knobs (resolved by experiments; see explore.py):
#   tt1/tt2: engine for the two elementwise passes ("vector" | "gpsimd")
#   reduce: "act" (ScalarE activation-accum per block), "dve" (one
#           tensor_reduce per group), or "split" (alternate groups)
#   rings:  1 = all input DMAs on the sync HWDGE ring; 2 = dst on scalar ring
#   inplace: write pass-1 output into the src tile (no separate prod pool)
#   layout: "rowmajor" (partition = row % 128; needs PE-transpose of scores)
#           "colmajor" (partition p owns rows [p*C, (p+1)*C); contiguous
#           8KB-per-partition DMA descriptors and a single contiguous
#           output DMA, no transpose)
#   cast:   None = fp32 tiles (HWDGE); "dst" = dst loaded via SWDGE
#           (gpsimd) with fp32->bf16 cast so pass2 + reduce run in DVE 2x
#           packed mode; "both" = src too (pass1 also 2x).
#   scores_bf16: reduce writes bf16 scores (one more 2^-9 rounding;
#           keeps tensor_reduce in 2x mode); a DVE copy upcasts to f32
#           before the plain HWDGE output DMA.
#   out_chunk: flush completed scores columns every N columns so the
#           upcast+output DMA overlaps the stream instead of draining.
DEFAULT_CFG = dict(
    tt1="vector", tt2="vector", reduce="dve", rings=1, inplace=True,
    bufs_io=6, bufs_pr=3, layout="colmajor", tail_first=True,
    cast="both", scores_bf16=True, out_chunk=0,
)
# out_chunk=128 measured perf-neutral (within +-1us) and produced one
# rare 2.6e-1 correctness failure (timing-dependent flush race) in a
# full test.py run; keep the single strictly-ordered end flush.

_CACHE = {}


def _build(rows, group, repeat=1, cfg=None, bench_internal=False):
    """Build the per-core program. repeat>1 wraps the whole body in a HW
    loop — used only for wall-clock benchmarking (amortizes dispatch
    overhead); the graded kernel uses repeat=1.

    bench_internal=True makes src/dst Internal DRAM tensors (zero-filled
    once before the repeat loop) so a benchmark dispatch doesn't ship
    512MB through the axon tunnel — only rel is an input. Timing is
    identical: same HBM traffic, same instruction stream."""
    import contextlib

    import concourse.bacc as bacc
    import concourse.tile as tile
    from concourse import masks, mybir

    cfg = {**DEFAULT_CFG, **(cfg or {})}
    f32 = mybir.dt.float32

    nc = bacc.Bacc(
        "TRN2", target_bir_lowering=False, debug=False, num_devices=N_CORES
    )
    kind_in = "Internal" if bench_internal else "ExternalInput"
    src = nc.dram_tensor("src", [rows, D], f32, kind=kind_in).ap()
    dst = nc.dram_tensor("dst", [rows, D], f32, kind=kind_in).ap()
    rel = nc.dram_tensor("rel", [1, D], f32, kind="ExternalInput").ap()
    out = nc.dram_tensor("out", [rows], f32, kind="ExternalOutput").ap()

    n_blocks = rows // P
    tail = rows - n_blocks * P

    with tile.TileContext(nc) as tc:
        with (
            tc.tile_pool(name="io", bufs=cfg["bufs_io"]) as io_pool,
            tc.tile_pool(name="prod", bufs=cfg["bufs_pr"]) as prod_pool,
            tc.tile_pool(name="consts", bufs=1) as consts,
            tc.tile_pool(
                name="scorep", bufs=cfg.get("scores_bufs", 1)
            ) as scorep,
            tc.tile_pool(name="psum", bufs=2, space="PSUM") as psum_pool,
        ):
            bf16 = mybir.dt.bfloat16
            rel_dt = (
                bf16 if cfg.get("cast") in ("src", "both", "dsrc") else f32
            )
            relrep = consts.tile([P, D], rel_dt)
            if rel_dt == f32:
                nc.sync.dma_start(
                    out=relrep[:], in_=rel[0:1, :].broadcast_to([P, D])
                )
            else:
                # broadcast in f32 (known-good DRAM->SBUF partition
                # broadcast), then DVE cast-copy to bf16
                rel_f32 = consts.tile([P, D], f32)
                nc.sync.dma_start(
                    out=rel_f32[:], in_=rel[0:1, :].broadcast_to([P, D])
                )
                nc.vector.tensor_copy(relrep[:], rel_f32[:])
            dummy = consts.tile([P, D], f32)
            if cfg["layout"] == "colmajor":
                ident = None  # PE transpose unused; skip startup cost
            else:
                ident = consts.tile([P, P], f32)
                masks.make_identity(nc, ident[:])
            if cfg["reduce"] == "mixo":
                cfg["_consts_pool"] = consts
                score_cols = (
                    cfg.get("nd_blocks")
                    or int(round(n_blocks * 0.5 / group)) * group
                )
            else:
                score_cols = n_blocks + (
                    1 if (tail and not cfg.get("scores_bf16")) else 0
                )
            cfg["_consts_pool"] = consts
            cfg["_scorep"] = scorep
            cfg["_score_cols"] = score_cols
            cfg["_scores_dt"] = (
                bf16
                if cfg.get("scores_bf16") and not cfg.get("nocompute")
                else f32
            )

            if bench_internal:
                # Zero-fill src/dst once (outside the repeat loop) so the
                # benched body never reads uninitialized HBM (NaN/denorm
                # timing hazards). ~36ms one-time vs the repeat loop.
                zt = consts.tile([P, 2048], f32)
                nc.gpsimd.memset(zt[:], 0.0)
                for t in (src, dst):
                    flat = t[0 : P * n_blocks, :].rearrange(
                        "(p c) d -> p (c d)", p=P
                    )
                    ncols = flat.shape[1]
                    for c0 in range(0, ncols, 2048):
                        cw = min(2048, ncols - c0)
                        nc.sync.dma_start(
                            out=flat[:, c0 : c0 + cw], in_=zt[:, 0:cw]
                        )
                    if tail:
                        nc.sync.dma_start(
                            out=t[n_blocks * P : rows, :],
                            in_=zt[0:tail, 0:D],
                        )

            # The For_i back-edge is an all-engine barrier (~2us) plus a
            # possible IRAM refetch stall — every iteration fully drains
            # the pipeline. For measurement (repeat>1), unroll several
            # kernel bodies per loop iteration so consecutive executions
            # pipeline and the back-edge cost amortizes; hint_engines
            # arms branch prefetch for the large unrolled body.
            unroll = max(1, cfg.get("unroll", 1)) if repeat > 1 else 1
            if repeat > 1:
                assert repeat % unroll == 0, (repeat, unroll)
                hints = (
                    (
                        mybir.EngineType.SP,
                        mybir.EngineType.Activation,
                        mybir.EngineType.DVE,
                        mybir.EngineType.PE,
                        mybir.EngineType.Pool,
                    )
                    if cfg.get("hints", unroll > 1)
                    else ()
                )
                loop = tc.For_i(0, repeat // unroll, 1, hint_engines=hints)
            else:
                loop = contextlib.nullcontext()
            with loop:
                for _u in range(unroll):
                    scores = scorep.tile(
                        [P, score_cols], cfg["_scores_dt"], name="scores"
                    )
                    _emit_body(
                        nc, tc, io_pool, prod_pool, psum_pool,
                        src, dst, out, relrep, dummy, ident, scores,
                        rows, group, n_blocks, tail, f32, cfg,
                    )
    nc.compile()
    return nc


def _emit_mixo(
    nc, tc, io_pool, prod_pool, psum_pool,
    src, dst, out, relrep, dummy, ident, scores,
    rows, group, n_blocks, tail, f32, cfg,
):
    """Mixed-engine reduce with contiguous per-engine column ownership.

    DVE owns blocks [0, ND), ACT owns [ND, n_blocks). Chunks of `group`
    blocks are processed in an interleaved order (D0 A0 D1 A1 ...) so both
    engines stream concurrently, but each engine's scores land in its own
    tile with a contiguous column range -> no cross-engine writes to one
    tile, and the two output DMAs stay contiguous (>=512B runs).
    """
    from concourse import mybir

    mult = mybir.AluOpType.mult
    nb = n_blocks
    g = group
    nd = cfg.get("nd_blocks") or int(round(nb * 0.5 / g)) * g
    na = nb - nd
    scoresD = scores  # [P, nd]
    consts_pool = cfg["_consts_pool"]
    scoresA = consts_pool.tile([P, na], f32, name="scoresA", tag="scoresA")
    scoresT = (
        consts_pool.tile([P, 1], f32, name="scoresT", tag="scoresT")
        if tail
        else None
    )

    src_v = src[0 : P * nb, :].rearrange("(p c) d -> p c d", p=P)
    dst_v = dst[0 : P * nb, :].rearrange("(p c) d -> p c d", p=P)

    def emit_tail():
        st = io_pool.tile([tail, D], f32, tag="s_tail")
        dt_ = io_pool.tile([tail, D], f32, tag="d_tail")
        nc.sync.dma_start(out=st[:], in_=src[nb * P : rows, :])
        nc.sync.dma_start(out=dt_[:], in_=dst[nb * P : rows, :])
        prt = prod_pool.tile([tail, D], f32, tag="pr_tail")
        nc.vector.tensor_tensor(prt[:], st[:], relrep[0:tail, :], mult)
        nc.vector.tensor_tensor(prt[:], prt[:], dt_[:], mult)
        nc.scalar.activation(
            dummy[0:tail, :],
            prt[:],
            mybir.ActivationFunctionType.Copy,
            accum_out=scoresT[0:tail, 0:1],
        )

    nocompute = cfg.get("nocompute", False)
    if tail and not nocompute:
        emit_tail()

    # chunk lists: (c0, cw, engine)
    chunksD = [(c0, min(g, nd - c0)) for c0 in range(0, nd, g)]
    chunksA = [(c0, min(g, nb - c0)) for c0 in range(nd, nb, g)]
    order = []
    i = j = 0
    while i < len(chunksD) or j < len(chunksA):
        if i < len(chunksD):
            order.append((*chunksD[i], "dve"))
            i += 1
        if j < len(chunksA):
            order.append((*chunksA[j], "act"))
            j += 1

    dcol = 0
    acol = 0
    for c0, cw, engine in order:
        s = io_pool.tile([P, g, D], f32, tag="s")
        d_ = io_pool.tile([P, g, D], f32, tag="d")
        nc.sync.dma_start(out=s[:, 0:cw, :], in_=src_v[:, c0 : c0 + cw, :])
        nc.sync.dma_start(out=d_[:, 0:cw, :], in_=dst_v[:, c0 : c0 + cw, :])
        if nocompute:
            nc.vector.tensor_copy(dummy[:, 0:1], s[:, 0, 0:1])
            nc.vector.tensor_copy(dummy[:, 1:2], d_[:, 0, 0:1])
            continue
        pr = s
        relb = relrep[:].unsqueeze(1).broadcast_to([P, cw, D])
        nc.vector.tensor_tensor(pr[:, 0:cw, :], s[:, 0:cw, :], relb, mult)
        nc.vector.tensor_tensor(pr[:, 0:cw, :], pr[:, 0:cw, :], d_[:, 0:cw, :], mult)
        if engine == "dve":
            nc.vector.tensor_reduce(
                scoresD[:, dcol : dcol + cw],
                pr[:, 0:cw, :],
                axis=mybir.AxisListType.X,
                op=mybir.AluOpType.add,
            )
            dcol += cw
        else:
            for jj in range(cw):
                nc.scalar.activation(
                    dummy[:],
                    pr[:, jj, :],
                    mybir.ActivationFunctionType.Copy,
                    accum_out=scoresA[:, acol : acol + 1],
                )
                acol += 1

    vout = out[0 : P * nb].rearrange("(p c) -> p c", p=P)
    if not nocompute:
        nc.scalar.dma_start(out=vout[:, 0:nd], in_=scoresD[:, 0:nd])
        nc.scalar.dma_start(out=vout[:, nd:nb], in_=scoresA[:, 0:na])
        if tail:
            nc.scalar.dma_start(
                out=out[nb * P : rows].rearrange("(p x) -> p x", x=1),
                in_=scoresT[0:tail, 0:1],
            )
    else:
        # output still must be written: initialize a sliver of the score
        # tiles (so every written tile has a reader and vice versa) and
        # dump the (garbage) scores. Floor-measurement only.
        nc.vector.tensor_copy(scoresD[:, 0:2], dummy[:, 0:2])
        nc.vector.tensor_copy(scoresA[:, 0:2], dummy[:, 0:2])
        nc.scalar.dma_start(out=vout[:, 0:nd], in_=scoresD[:, 0:nd])
        nc.scalar.dma_start(out=vout[:, nd:nb], in_=scoresA[:, 0:na])


def _flush_out(nc, cfg, scores, out, a, b, n_blocks, f32, scores_bf16):
    """Upcast+DMA the completed scores column range [a, b)."""
    vout = out[0 : P * n_blocks].rearrange("(p c) -> p c", p=P)
    if scores_bf16:
        stage = cfg["_scorep"].tile(
            [P, b - a], f32, name="oflush", tag="oflush"
        )
        nc.vector.tensor_copy(stage[:], scores[:, a:b])
        nc.scalar.dma_start(out=vout[:, a:b], in_=stage[:])
    else:
        nc.scalar.dma_start(out=vout[:, a:b], in_=scores[:, a:b])


def _emit_body(
    nc, tc, io_pool, prod_pool, psum_pool,
    src, dst, out, relrep, dummy, ident, scores,
    rows, group, n_blocks, tail, f32, cfg,
):
    from concourse import mybir

    cfg["_out_flushed"] = 0
    if cfg["reduce"] == "mixo":
        _emit_mixo(
            nc, tc, io_pool, prod_pool, psum_pool,
            src, dst, out, relrep, dummy, ident, scores,
            rows, group, n_blocks, tail, f32, cfg,
        )
        return

    nocompute = cfg.get("nocompute", False)
    mult = mybir.AluOpType.mult
    eng = {"vector": nc.vector, "gpsimd": nc.gpsimd}
    tt1 = eng[cfg["tt1"]]
    tt2 = eng[cfg["tt2"]]
    # rings: 1 = all input DMAs on sync HWDGE; 2 = dst on scalar HWDGE
    # (bad when ACT also reduces); 3 = dst on gpsimd SWDGE (gpsimd idle)
    dma_d = {1: nc.sync, 2: nc.scalar, 3: nc.gpsimd}[cfg["rings"]]
    colmajor = cfg["layout"] == "colmajor"
    bf16 = mybir.dt.bfloat16
    cast = cfg.get("cast")
    # "dsrc": src arrives fp32 on a HWDGE ring and is cast to bf16 by a
    # DVE copy (2x_2p) — splits input DMA across the sync HWDGE row and
    # the SWDGE row while keeping the all-bf16 compute pipeline.
    s_dt = bf16 if cast in ("src", "both") else f32
    d_dt = bf16 if cast in ("dst", "both", "dsrc") else f32
    pr_dt = bf16 if cast else f32
    # Ring selection: s_ring/d_ring in {"sync","scalar","alt","gp"}.
    # "alt" alternates sync/scalar per group (2 HWDGE queue rows);
    # "gp" is the SWDGE row (required for cast loads).
    ring_map = {"sync": nc.sync, "scalar": nc.scalar, "gp": nc.gpsimd}
    s_ring = cfg.get("s_ring", "gp" if cast in ("src", "both") else "sync")
    d_ring = cfg.get(
        "d_ring",
        "gp" if cast in ("dst", "both", "dsrc") else
        {1: "sync", 2: "scalar", 3: "gp"}[cfg["rings"]],
    )
    if cast in ("dst", "both", "dsrc"):
        assert d_ring == "gp"
    if cast in ("src", "both"):
        assert s_ring == "gp"
    if cast == "dsrc":
        assert s_ring != "gp"

    def ring_eng(which, gi):
        if which == "alt":
            return nc.sync if gi % 2 == 0 else nc.scalar
        return ring_map[which]

    dma_s = ring_map.get(s_ring, nc.sync)
    dma_d = ring_map.get(d_ring, nc.sync)
    scores_dt = cfg["_scores_dt"]
    scores_bf16 = scores_dt != f32
    if scores_bf16:
        assert cfg["reduce"] == "dve", "scores_bf16 needs reduce=dve"
        scoresT = (
            cfg["_consts_pool"].tile([P, 1], f32, name="scoresT", tag="scoresT")
            if tail
            else None
        )
    import contextlib

    def lowp():
        fn = getattr(nc, "allow_low_precision", None)
        if scores_bf16 and fn is not None:
            return fn(reason="bf16 scores within 2e-2 gate")
        return contextlib.nullcontext()
    if colmajor:
        # [P*n_blocks, D] viewed so partition p owns rows [p*n_blocks, ...)
        src_v = src[0 : P * n_blocks, :].rearrange("(p c) d -> p c d", p=P)
        dst_v = dst[0 : P * n_blocks, :].rearrange("(p c) d -> p c d", p=P)

    def emit_tail():
        st = io_pool.tile([tail, D], f32, tag="s_tail")
        dt_ = io_pool.tile([tail, D], f32, tag="d_tail")
        nc.sync.dma_start(out=st[:], in_=src[n_blocks * P : rows, :])
        nc.sync.dma_start(out=dt_[:], in_=dst[n_blocks * P : rows, :])
        prt = prod_pool.tile([tail, D], f32, tag="pr_tail")
        nc.vector.tensor_tensor(prt[:], st[:], relrep[0:tail, :], mult)
        nc.vector.tensor_tensor(prt[:], prt[:], dt_[:], mult)
        tail_out = (
            scoresT[0:tail, 0:1]
            if scores_bf16
            else scores[0:tail, n_blocks : n_blocks + 1]
        )
        nc.scalar.activation(
            dummy[0:tail, :],
            prt[:],
            mybir.ActivationFunctionType.Copy,
            accum_out=tail_out,
        )

    if tail and cfg["tail_first"] and not nocompute:
        emit_tail()

    col = 0
    b0 = 0
    gi = 0
    sub_gi = 0
    while b0 < n_blocks:
        g = min(group, n_blocks - b0)
        s = io_pool.tile([P, g, D], s_dt, tag="s")
        d_ = io_pool.tile([P, g, D], d_dt, tag="d")
        if colmajor:
            in_s = src_v[:, b0 : b0 + g, :]
            in_d = dst_v[:, b0 : b0 + g, :]
        else:
            in_s = src[b0 * P : (b0 + g) * P, :].rearrange(
                "(j p) d -> p j d", p=P
            )
            in_d = dst[b0 * P : (b0 + g) * P, :].rearrange(
                "(j p) d -> p j d", p=P
            )
        ring_eng(s_ring, gi).dma_start(out=s[:], in_=in_s)
        ring_eng(d_ring, gi + 1).dma_start(out=d_[:], in_=in_d)
        if nocompute:
            nc.vector.tensor_copy(dummy[:, 0:1], s[:, 0, 0:1])
            nc.vector.tensor_copy(dummy[:, 1:2], d_[:, 0, 0:1])
            b0 += g
            gi += 1
            continue
        if cast == "dsrc":
            s_use = prod_pool.tile([P, g, D], bf16, tag="sbf")
            nc.vector.tensor_copy(s_use[:], s[:])
        else:
            s_use = s
        if cfg["inplace"] and (s_use is not s or s_dt == pr_dt):
            pr = s_use
        else:
            pr = prod_pool.tile([P, g, D], pr_dt, tag="pr")
        relb = relrep[:].unsqueeze(1).broadcast_to([P, g, D])
        mode = cfg["reduce"]
        if mode == "split":
            mode = "act" if gi % 2 == 0 else "dve"
        tt1.tensor_tensor(pr[:], s_use[:], relb, mult)
        if mode != "ttr":
            tt2.tensor_tensor(pr[:], pr[:], d_[:], mult)
        if mode == "ttr":
            # Fused multiply+reduce on DVE: one instruction per block does
            # scratch = pr * dst and scores[:, col] = sum_d(scratch) — no
            # ACT needed. (ttr_inplace=True writes back into pr instead of
            # the dummy scratch; that variant wedged the device once.)
            for j in range(g):
                ttr_out = pr[:, j, :] if cfg.get("ttr_inplace") else dummy[:]
                nc.vector.tensor_tensor_reduce(
                    out=ttr_out,
                    in0=pr[:, j, :],
                    in1=d_[:, j, :],
                    scale=1.0,
                    scalar=0.0,
                    op0=mult,
                    op1=mybir.AluOpType.add,
                    accum_out=scores[:, col : col + 1],
                )
                col += 1
        elif mode == "split8":
            # Alternating ACT/DVE reduce in 8-block (32B-aligned)
            # subchunks, independent of the DMA group size — combines
            # large-descriptor DMA with the fine engine interleave that
            # the g=8 alternating split measured best.
            for sc in range(0, g, 8):
                scw = min(8, g - sc)
                if sub_gi % 2 == 0:
                    for j in range(sc, sc + scw):
                        nc.scalar.activation(
                            dummy[:],
                            pr[:, j, :],
                            mybir.ActivationFunctionType.Copy,
                            accum_out=scores[:, col : col + 1],
                        )
                        col += 1
                else:
                    nc.vector.tensor_reduce(
                        scores[:, col : col + scw],
                        pr[:, sc : sc + scw, :],
                        axis=mybir.AxisListType.X,
                        op=mybir.AluOpType.add,
                    )
                    col += scw
                sub_gi += 1
        elif mode == "gmix":
            # Within-group split: DVE tensor_reduces the first gd blocks in
            # one instruction, GPSIMD the next gg, ACT accumulates the rest
            # — balanced every group, no cross-group engine alternation.
            # Keep all boundaries multiples of 8 columns (32B) — unaligned
            # ranges in the shared scores tile create false dependencies.
            gd = min(cfg.get("gmix_dve", (g + 1) // 2), g)
            gg = min(cfg.get("gmix_gp", 0), g - gd)
            if gd > 0:
                nc.vector.tensor_reduce(
                    scores[:, col : col + gd],
                    pr[:, 0:gd, :],
                    axis=mybir.AxisListType.X,
                    op=mybir.AluOpType.add,
                )
            if gg > 0:
                nc.gpsimd.tensor_reduce(
                    scores[:, col + gd : col + gd + gg],
                    pr[:, gd : gd + gg, :],
                    axis=mybir.AxisListType.X,
                    op=mybir.AluOpType.add,
                )
            for j in range(gd + gg, g):
                nc.scalar.activation(
                    dummy[:],
                    pr[:, j, :],
                    mybir.ActivationFunctionType.Copy,
                    accum_out=scores[:, col + j : col + j + 1],
                )
            col += g
        elif mode == "act":
            for j in range(g):
                nc.scalar.activation(
                    dummy[:],
                    pr[:, j, :],
                    mybir.ActivationFunctionType.Copy,
                    accum_out=scores[:, col : col + 1],
                )
                col += 1
        else:  # dve
            with lowp():
                nc.vector.tensor_reduce(
                    scores[:, col : col + g],
                    pr[:],
                    axis=mybir.AxisListType.X,
                    op=mybir.AluOpType.add,
                )
            col += g
        b0 += g
        gi += 1
        # Chunked output: as soon as a column chunk of scores is
        # complete, upcast + DMA it out — hides the output path in the
        # steady state instead of serializing it into the drain.
        ocw = cfg.get("out_chunk", 0)
        if ocw and colmajor and not nocompute:
            done = (col // ocw) * ocw
            flushed = cfg.setdefault("_out_flushed", 0)
            if b0 >= n_blocks:
                done = n_blocks  # final flush covers the remainder
            if done > flushed:
                _flush_out(
                    nc, cfg, scores, out, flushed, done, n_blocks, f32,
                    scores_bf16,
                )
                cfg["_out_flushed"] = done

    if tail and not cfg["tail_first"] and not nocompute:
        emit_tail()

    if nocompute:
        nc.vector.tensor_copy(scores[:, 0:2], dummy[:, 0:2])
        nc.scalar.dma_start(
            out=out[0 : P * n_blocks].rearrange("(p c) -> p c", p=P),
            in_=scores[:, 0:n_blocks],
        )
        return

    if colmajor:
        # scores[p, c] is already the score of row p*n_blocks + c:
        # contiguous output DMA(s). (bf16 scores upcast via DVE copy —
        # ~300ns — then plain HWDGE DMA.) With out_chunk set, the
        # columns were already flushed incrementally above.
        if cfg.get("_out_flushed", 0) < n_blocks:
            _flush_out(
                nc, cfg, scores, out, cfg.get("_out_flushed", 0),
                n_blocks, n_blocks, f32, scores_bf16,
            )
    else:
        # Transpose score columns so output DMAs are contiguous per row.
        for c0 in range(0, n_blocks, P):
            cc = min(P, n_blocks - c0)
            pt = psum_pool.tile([P, P], f32, tag="pt")
            nc.tensor.transpose(pt[0:cc, :], scores[:, c0 : c0 + cc], ident[:])
            sb = prod_pool.tile([P, P], f32, tag="scoresT")
            nc.vector.tensor_copy(sb[0:cc, :], pt[0:cc, :])
            nc.scalar.dma_start(
                out=out[c0 * P : (c0 + cc) * P].rearrange("(t p) -> t p", p=P),
                in_=sb[0:cc, :],
            )
    if tail:
        tail_src = (
            scoresT[0:tail, 0:1]
            if scores_bf16
            else scores[0:tail, n_blocks : n_blocks + 1]
        )
        nc.scalar.dma_start(
            out=out[n_blocks * P : rows].rearrange("(p x) -> p x", x=1),
            in_=tail_src,
        )


def _get_program(rows, group):
    key = (rows, group)
    if key not in _CACHE:
        _CACHE[key] = _build(rows, group)
    return _CACHE[key]


def kernel(src_emb, dst_emb, M, rel_idx):
    from concourse.bass_utils import run_bass_kernel_spmd

    src_emb = np.asarray(src_emb, dtype=np.float32)
    dst_emb = np.asarray(dst_emb, dtype=np.float32)
    M = np.asarray(M, dtype=np.float32)
    rel = np.ascontiguousarray(M[int(rel_idx)]).reshape(1, D)

    nc = _get_program(ROWS, GROUP)
    in_maps = [
        {
            "src": np.ascontiguousarray(src_emb[i * ROWS : (i + 1) * ROWS]),
            "dst": np.ascontiguousarray(dst_emb[i * ROWS : (i + 1) * ROWS]),
            "rel": rel,
        }
        for i in range(N_CORES)
    ]
    res = run_bass_kernel_spmd(nc, in_maps, list(range(N_CORES)))
    out = np.concatenate([res.results[i]["out"] for i in range(N_CORES)])
    return out.reshape(B, 1, 1).astype(np.float32)

